# revision 40
# baseline (speedup 1.0000x reference)
"""MinGRU (2-layer) Trainium2 Bass kernel.

Problem: h[8,4096,1024] f32, W0/W1 [1024,3072] f32.
Per layer: z = h @ W; hidden,gate,proj = split(z);
  a = sigmoid(-gate); g_hidden = relu(hidden) + min(sigmoid(hidden), 0.5)
  scan: out_t = a_t*out_{t-1} + (1-a_t)*g_hidden_t   (fp32 scan state)
  h' = sigmoid(proj)*out + (1-sigmoid(proj))*h

Sharding: one batch row per core (B=8 over 8 cores), weights replicated.

Design (engine-balanced against measured per-op HW costs):
  - hidden matmul fp16 (accuracy-critical path); gate/proj matmuls fp8 e4m3
    with DoubleRow perf mode (2 k-tiles per instruction, 2x PE throughput).
    Measured rel err 1.28e-2 vs the 2e-2 gate on the fixed harness inputs.
  - host pre-transposes h to [H,T] fp16+fp8(x8); y is written [H,T] fp16 and
    the host re-transposes + upcasts. No PE or DMA transposes on device.
  - PSUM tiles span 2 banks [128,1024]; ACT reads a full span in one op
    (1.18us vs 2x0.91us measured) and writes fp16 SBUF tiles.
  - elementwise split across engines per 1024-token span:
      ACT:  s=sig(hidden), r=relu(hidden), a=sig(-gate), g=sig(proj), fp8 cast
      DVE:  gh=min(s,.5)+r, negb=(a-1)*gh, tensor_tensor_scan, h'=m+h, carry
      Pool: d=sc-h, m=g*d
  - layer-2 interleaves with layer-1 one span behind; the DVE highway-out
    runs 2 units behind its producer and the ACT cast / y-DMA 2 units behind
    (popped after the DVE tail) so the in-order engine streams never block
    on the cross-engine scan->Pool chain.
  - input DMAs issue from the Pool queue (25ns/issue vs 565ns on SP).

Measured: ~630-650 us HW exec (baseline 914 us; shared-device load
adds up to +15% run-to-run), rel err 1.275e-2.
"""

import os
import sys

if "/opt/trn_rl_repo" not in sys.path:
    sys.path.insert(0, "/opt/trn_rl_repo")

from contextlib import ExitStack

import numpy as np
import ml_dtypes

import concourse.bass as bass
import concourse.tile as tile
from concourse import bacc, mybir
from concourse import bass_utils


T, H, H3 = 4096, 1024, 3072
TCE = 1024               # elementwise span (= 2 PSUM banks of fp32)
TSUB = 512               # DVE/Pool subtile
NSPAN = T // TCE
NFB = H // 128           # output feature blocks
NK = H // 128            # contraction k-tiles
F32 = mybir.dt.float32
F16 = mybir.dt.float16
F8 = mybir.dt.float8e4
ACT = mybir.ActivationFunctionType
ALU = mybir.AluOpType
DR = mybir.MatmulPerfMode.DoubleRow

SH = 8.0                 # fp8 activation scale
SW = 32.0                # fp8 weight scale
INV8 = 1.0 / (SH * SW)
C16 = H                  # fp16 weight cols per layer (hidden)
C8 = 2 * H               # fp8 weight cols per layer (gate, proj)


def _emit_unit(nc, i, li, f, w16_sb, w8_sb, rhs16, rhs8, dst16, dst8,
               psums, ew, carries, y16):
    """Emit matmuls + front elementwise for one (span, layer, f-block).

    Returns a closure emitting the tail (DVE highway-out, then ACT fp8 cast
    or the y DMA) which the caller schedules 1-2 units later so the in-order
    ACT/DVE streams never block on the cross-engine scan->Pool chain.
    """
    psum_h, psum_g, psum_p = psums
    ph = psum_h.tile([128, TCE], F32, tag="ph")
    pg = psum_g.tile([128, TCE], F32, tag="pg")
    pp = psum_p.tile([128, TCE], F32, tag="pp")
    w16 = w16_sb[li]
    w8 = w8_sb[li]
    for half in (0, 1):
        sl = slice(half * 512, (half + 1) * 512)
        for k in range(NK):
            nc.tensor.matmul(ph[:, sl], w16[:, k, f * 128:(f + 1) * 128],
                             rhs16[:, k, sl],
                             start=(k == 0), stop=(k == NK - 1))
        for k in range(0, NK, 2):
            nc.tensor.matmul(pg[:, sl], w8[:, k:k + 2, f * 128:(f + 1) * 128],
                             rhs8[:, k:k + 2, sl], perf_mode=DR,
                             start=(k == 0), stop=(k == NK - 2))
        for k in range(0, NK, 2):
            nc.tensor.matmul(pp[:, sl],
                             w8[:, k:k + 2, H + f * 128:H + (f + 1) * 128],
                             rhs8[:, k:k + 2, sl], perf_mode=DR,
                             start=(k == 0), stop=(k == NK - 2))
    # ACT: full-span transcendentals out of PSUM (s/r first: they gate the
    # next unit's hidden matmul group via psum reuse)
    s_ = ew.tile([128, TCE], F16, tag="s")
    if os.environ.get("ABLATE") == "2":
        nc.scalar.activation(s_[:, 0:1], ph[:, 0:1], ACT.Sigmoid)
        nc.scalar.activation(s_[:, 1:2], pg[:, 0:1], ACT.Sigmoid)
        nc.scalar.activation(s_[:, 2:3], pp[:, 0:1], ACT.Sigmoid)

        def tail_dve():
            pass

        def tail_fin():
            if li == 0:
                nc.scalar.activation(dst8[:, f, 0:1], s_[:, 0:1], ACT.Copy,
                                     scale=SH)
                nc.vector.tensor_copy(dst16[:, f, 0:1], s_[:, 0:1])
            else:
                nc.sync.dma_start(
                    y16[f * 128:(f + 1) * 128, i * TCE:(i + 1) * TCE], s_[:])
        return tail_dve, tail_fin
    nc.scalar.activation(s_[:], ph[:], ACT.Sigmoid)
    r_ = ew.tile([128, TCE], F16, tag="r")
    nc.scalar.activation(r_[:], ph[:], ACT.Relu)
    a_ = ew.tile([128, TCE], F16, tag="a")
    nc.scalar.activation(a_[:], pg[:], ACT.Sigmoid, scale=-INV8)
    g_ = ew.tile([128, TCE], F16, tag="g")
    nc.scalar.activation(g_[:], pp[:], ACT.Sigmoid, scale=INV8)

    col = li * NFB + f
    if os.environ.get("ABLATE") == "1":
        # timing ablation: no DVE/Pool elementwise; cast/dst fed from s_
        def tail_dve():
            pass

        def tail_fin():
            if li == 0:
                nc.scalar.activation(dst8[:, f, :], s_[:], ACT.Copy, scale=SH)
                nc.vector.tensor_copy(dst16[:, f, :], s_[:])
            else:
                nc.sync.dma_start(
                    y16[f * 128:(f + 1) * 128, i * TCE:(i + 1) * TCE], s_[:])
        return tail_dve, tail_fin
    gh = ew.tile([128, TCE], F16, tag="gh")
    nc.vector.scalar_tensor_tensor(gh[:], s_[:], 0.5, r_[:],
                                   op0=ALU.min, op1=ALU.add)
    negb = ew.tile([128, TCE], F16, tag="nb")
    nc.vector.scalar_tensor_tensor(negb[:], a_[:], 1.0, gh[:],
                                   op0=ALU.subtract, op1=ALU.mult)
    sc = ew.tile([128, TCE], F16, tag="sc")
    init = 0.0 if i == 0 else carries[:, col:col + 1]
    if os.environ.get("ABLATE") == "4":
        # timing-only: same-shape tt instead of the scan
        nc.vector.tensor_tensor(sc[:], a_[:], negb[:], op=ALU.mult)
    else:
        nc.vector.tensor_tensor_scan(sc[:], a_[:], negb[:], init,
                                     op0=ALU.mult, op1=ALU.subtract)
    if i < NSPAN - 1:
        nc.vector.tensor_copy(carries[:, col:col + 1], sc[:, TCE - 1:TCE])
    # Pool runs the whole highway chain in-order on one engine: no
    # cross-engine re-entry into the in-order ACT/DVE streams.
    hs = rhs16[:, f, :]
    d_ = ew.tile([128, TCE], F16, tag="d")
    if os.environ.get("POOL_DM", "1") == "1":
        nc.gpsimd.tensor_tensor(d_[:], sc[:], hs, op=ALU.subtract)
    else:
        nc.vector.tensor_tensor(d_[:], sc[:], hs, op=ALU.subtract)
    m_ = ew.tile([128, TCE], F16, tag="m", bufs=3)
    if os.environ.get("POOL_DM", "1") == "1":
        nc.gpsimd.tensor_tensor(m_[:], g_[:], d_[:], op=ALU.mult)
    else:
        nc.vector.tensor_tensor(m_[:], g_[:], d_[:], op=ALU.mult)

    def tail_dve():
        nc.vector.tensor_tensor(dst16[:, :] if li else dst16[:, f, :],
                                m_[:], hs, op=ALU.add)

    def tail_fin():
        if li == 0:
            nc.scalar.activation(dst8[:, f, :], dst16[:, f, :], ACT.Copy,
                                 scale=SH)
        else:
            nc.sync.dma_start(
                y16[f * 128:(f + 1) * 128, i * TCE:(i + 1) * TCE],
                dst16[:, :])

    return tail_dve, tail_fin


def _emit_body(tc_, y16, h16t, h8t, w16_sb, w8_sb, pools):
    nc = tc_.nc
    rhs_pool, ypool, psums, ew, carry_pool = pools
    carries = carry_pool.tile([128, 2 * NFB], F32)

    # Software-pipelined tails: the ACT fp8-cast / y-DMA runs 3 units
    # behind its producer so the in-order ACT/SP streams never block on the
    # scan->Pool highway chain.
    pend_dve = []
    pend_fin = []

    def emit(unit_args):
        # front first: the ACT sigmoids (which feed DVE) must precede the
        # delayed cast in the in-order ACT stream, else the loop-carried
        # cycle out->cast->s/r->gh->scan paces the whole kernel (~10us/unit
        # measured). Tag rotation stays safe: one allocation per unit per
        # tag, m has bufs=3, reads pop at most 2 units behind.
        td, tf = _emit_unit(*unit_args)
        if len(pend_dve) >= 2:
            pend_dve.pop(0)()
        if len(pend_fin) >= 2:
            pend_fin.pop(0)()
        pend_dve.append(td)
        pend_fin.append(tf)

    prev = None
    for i in range(NSPAN):
        rhs16 = rhs_pool.tile([128, NK, TCE], F16, tag="rhs16_l1")
        for k in range(NK):
            nc.gpsimd.dma_start(rhs16[:, k, :],
                                h16t[k * 128:(k + 1) * 128, i * TCE:(i + 1) * TCE])
        rhs8 = rhs_pool.tile([128, NK, TCE], F8, tag="rhs8_l1", bufs=1)
        for k in range(NK):
            nc.gpsimd.dma_start(rhs8[:, k, :],
                                h8t[k * 128:(k + 1) * 128, i * TCE:(i + 1) * TCE])
        out16 = rhs_pool.tile([128, NK, TCE], F16, tag="rhs16_l2")
        out8 = rhs_pool.tile([128, NK, TCE], F8, tag="rhs8_l2")
        if prev is None:
            for f in range(NFB):
                emit((nc, i, 0, f, w16_sb, w8_sb, rhs16, rhs8,
                      out16, out8, psums, ew, carries, None))
            # span 0 has no interleaved L2 units; flush so span 1's L2
            # matmuls see every span-0 cast already emitted
            for t in pend_dve:
                t()
            for t in pend_fin:
                t()
            pend_dve.clear()
            pend_fin.clear()
        else:
            (p16, p8) = prev
            for f in range(NFB):
                emit((nc, i, 0, f, w16_sb, w8_sb, rhs16, rhs8,
                      out16, out8, psums, ew, carries, None))
                ytile = ypool.tile([128, TCE], F16, tag="y", name="ytile")
                emit((nc, i - 1, 1, f, w16_sb, w8_sb, p16, p8,
                      ytile, None, psums, ew, carries, y16))
        prev = (out16, out8)
    (p16, p8) = prev
    # the final L2 block has no slack emit before its first unit: flush so
    # every span-3 cast/highway-out is emitted before L2 reads them
    for t in pend_dve:
        t()
    for t in pend_fin:
        t()
    pend_dve.clear()
    pend_fin.clear()
    for f in range(NFB):
        ytile = ypool.tile([128, TCE], F16, tag="y", name="ytile")
        emit((nc, NSPAN - 1, 1, f, w16_sb, w8_sb, p16, p8,
              ytile, None, psums, ew, carries, y16))
    for t in pend_dve:
        t()
    for t in pend_fin:
        t()


def build_nc(loop_iters: int = 1):
    """Build + compile the per-core Bass program (SPMD across 8 cores)."""
    nc = bacc.Bacc("TRN2", target_bir_lowering=False, debug=False,
                   enable_asserts=False, num_devices=8)
    h16t = nc.dram_tensor("h16t", [H, T], F16, kind="ExternalInput").ap()
    h8t = nc.dram_tensor("h8t", [H, T], F8, kind="ExternalInput").ap()
    w16 = nc.dram_tensor("w16", [2, NK, 128, C16], F16,
                         kind="ExternalInput").ap()
    w8 = nc.dram_tensor("w8", [2, NK, 128, C8], F8,
                        kind="ExternalInput").ap()
    y16 = nc.dram_tensor("y16", [H, T], F16, kind="ExternalOutput").ap()

    with tile.TileContext(nc) as tc_:
        with ExitStack() as ctx:
            wpool = ctx.enter_context(tc_.tile_pool(name="w", bufs=1))
            rhs_pool = ctx.enter_context(tc_.tile_pool(name="rhs", bufs=2))
            ypool = ctx.enter_context(tc_.tile_pool(name="y", bufs=2))
            psum_h = ctx.enter_context(
                tc_.tile_pool(name="psh", bufs=2, space="PSUM"))
            psum_g = ctx.enter_context(
                tc_.tile_pool(name="psg", bufs=1, space="PSUM"))
            psum_p = ctx.enter_context(
                tc_.tile_pool(name="psp", bufs=1, space="PSUM"))
            ew = ctx.enter_context(tc_.tile_pool(name="ew", bufs=2))
            carry_pool = ctx.enter_context(tc_.tile_pool(name="carry", bufs=1))

            w16_sb = []
            w8_sb = []
            for li in range(2):
                wt = wpool.tile([128, NK, C16], F16, tag=f"w16_{li}",
                                name=f"w16_{li}")
                for k in range(NK):
                    nc.gpsimd.dma_start(wt[:, k, :], w16[li, k])
                w16_sb.append(wt)
                wt8 = wpool.tile([128, NK, C8], F8, tag=f"w8_{li}",
                                 name=f"w8_{li}")
                for k in range(NK):
                    nc.gpsimd.dma_start(wt8[:, k, :], w8[li, k])
                w8_sb.append(wt8)

            # PE p-state warmup + ACT sigmoid table preload while the weight
            # stream is in flight. The warm matmuls write the proj psum tile
            # (reused by the first real unit afterwards).
            warm_in = ew.tile([128, TCE], F16, tag="gh", name="warm_in")
            nc.vector.memset(warm_in[:], 0.0)
            wp = psum_p.tile([128, TCE], F32, tag="pp", name="wp")
            for _ in range(24):
                nc.tensor.matmul(wp[:, 0:512], warm_in[:, 0:128],
                                 warm_in[:, 0:512], start=True, stop=True)
            warm_s = ew.tile([128, TCE], F16, tag="s", name="warm_s")
            nc.scalar.activation(warm_s[:, 0:1], wp[:, 0:1], ACT.Sigmoid)

            pools = (rhs_pool, ypool, (psum_h, psum_g, psum_p), ew, carry_pool)
            if loop_iters == 1:
                _emit_body(tc_, y16, h16t, h8t, w16_sb, w8_sb, pools)
            else:
                with tc_.For_i(0, loop_iters, 1):
                    _emit_body(tc_, y16, h16t, h8t, w16_sb, w8_sb, pools)
    nc.compile()
    return nc


_CACHED_NC = None


def _prep_inputs(h, W0, W1):
    e4 = ml_dtypes.float8_e4m3
    W = np.stack([np.asarray(W0, np.float32), np.asarray(W1, np.float32)])
    w16 = W[:, :, 0:H].reshape(2, NK, 128, C16)
    w8 = (W[:, :, H:] * SW).reshape(2, NK, 128, C8)
    base = {"w16": w16.astype(np.float16), "w8": w8.astype(e4)}
    maps = []
    for c in range(8):
        ht = np.ascontiguousarray(np.asarray(h[c]).T)
        m = dict(base)
        m["h16t"] = ht.astype(np.float16)
        m["h8t"] = (ht * SH).astype(e4)
        maps.append(m)
    return maps


def kernel(h, W0, W1):
    global _CACHED_NC
    if _CACHED_NC is None:
        _CACHED_NC = build_nc()
    res = bass_utils.run_bass_kernel_spmd(
        _CACHED_NC, _prep_inputs(h, W0, W1), core_ids=list(range(8)))
    return np.stack(
        [res.results[c]["y16"].T.astype(np.float32) for c in range(8)], axis=0)


# revision 41
# speedup vs baseline: 1.1013x; 1.1013x over previous
"""MinGRU (2-layer) Trainium2 Bass kernel.

Problem: h[8,4096,1024] f32, W0/W1 [1024,3072] f32.
Per layer: z = h @ W; hidden,gate,proj = split(z);
  a = sigmoid(-gate); g_hidden = relu(hidden) + min(sigmoid(hidden), 0.5)
  scan: out_t = a_t*out_{t-1} + (1-a_t)*g_hidden_t   (fp32 scan state)
  h' = sigmoid(proj)*out + (1-sigmoid(proj))*h

Sharding: one batch row per core (B=8 over 8 cores), weights replicated.

Design (engine-balanced against measured per-op HW costs):
  - hidden matmul fp16 (accuracy-critical path); gate/proj matmuls fp8 e4m3
    with DoubleRow perf mode (2 k-tiles per instruction, 2x PE throughput).
    Measured rel err 1.28e-2 vs the 2e-2 gate on the fixed harness inputs.
  - host pre-transposes h to [H,T] fp16+fp8(x8); y is written [H,T] fp16 and
    the host re-transposes + upcasts. No PE or DMA transposes on device.
  - PSUM tiles span 2 banks [128,1024]; ACT reads a full span in one op
    (1.18us vs 2x0.91us measured) and writes fp16 SBUF tiles.
  - elementwise split across engines per 1024-token span:
      ACT:  s=sig(hidden), r=relu(hidden), a=sig(-gate), g=sig(proj), fp8 cast
      DVE:  gh=min(s,.5)+r, negb=(a-1)*gh, tensor_tensor_scan, h'=m+h, carry
      Pool: d=sc-h, m=g*d
  - layer-2 interleaves with layer-1 one span behind; the DVE highway-out
    runs 2 units behind its producer and the ACT cast / y-DMA 2 units behind
    (popped after the DVE tail) so the in-order engine streams never block
    on the cross-engine scan->Pool chain.
  - input DMAs issue from the Pool queue (25ns/issue vs 565ns on SP).

Measured: ~630-650 us HW exec (baseline 914 us; shared-device load
adds up to +15% run-to-run), rel err 1.275e-2.
"""

import os
import sys

if "/opt/trn_rl_repo" not in sys.path:
    sys.path.insert(0, "/opt/trn_rl_repo")

from contextlib import ExitStack

import numpy as np
import ml_dtypes

import concourse.bass as bass
import concourse.tile as tile
from concourse import bacc, mybir
from concourse import bass_utils


T, H, H3 = 4096, 1024, 3072
TCE = 1024               # elementwise span (= 2 PSUM banks of fp32)
TSUB = 512               # DVE/Pool subtile
NSPAN = T // TCE
NFB = H // 128           # output feature blocks
NK = H // 128            # contraction k-tiles
F32 = mybir.dt.float32
F16 = mybir.dt.float16
F8 = mybir.dt.float8e4
ACT = mybir.ActivationFunctionType
ALU = mybir.AluOpType
DR = mybir.MatmulPerfMode.DoubleRow

SH = 8.0                 # fp8 activation scale
SW = 32.0                # fp8 weight scale
INV8 = 1.0 / (SH * SW)
C16 = H                  # fp16 weight cols per layer (hidden)
C8 = 2 * H               # fp8 weight cols per layer (gate, proj)


def _emit_unit(nc, i, li, f, w16_sb, w8_sb, rhs16, rhs8, dst16, dst8,
               psums, ew, carries, y16):
    """Emit matmuls + front elementwise for one (span, layer, f-block).

    Returns a closure emitting the tail (DVE highway-out, then ACT fp8 cast
    or the y DMA) which the caller schedules 1-2 units later so the in-order
    ACT/DVE streams never block on the cross-engine scan->Pool chain.
    """
    psum_h, psum_g, psum_p = psums
    ph = psum_h.tile([128, TCE], F32, tag="ph")
    pg = psum_g.tile([128, TCE], F32, tag="pg")
    pp = psum_p.tile([128, TCE], F32, tag="pp")
    w16 = w16_sb[li]
    w8 = w8_sb[li]
    for half in (0, 1):
        sl = slice(half * 512, (half + 1) * 512)
        for k in range(NK):
            nc.tensor.matmul(ph[:, sl], w16[:, k, f * 128:(f + 1) * 128],
                             rhs16[:, k, sl],
                             start=(k == 0), stop=(k == NK - 1))
        for k in range(0, NK, 2):
            nc.tensor.matmul(pg[:, sl], w8[:, k:k + 2, f * 128:(f + 1) * 128],
                             rhs8[:, k:k + 2, sl], perf_mode=DR,
                             start=(k == 0), stop=(k == NK - 2))
        for k in range(0, NK, 2):
            nc.tensor.matmul(pp[:, sl],
                             w8[:, k:k + 2, H + f * 128:H + (f + 1) * 128],
                             rhs8[:, k:k + 2, sl], perf_mode=DR,
                             start=(k == 0), stop=(k == NK - 2))
    # ACT: full-span transcendentals out of PSUM (s/r first: they gate the
    # next unit's hidden matmul group via psum reuse)
    s_ = ew.tile([128, TCE], F16, tag="s")
    if os.environ.get("ABLATE") == "2":
        nc.scalar.activation(s_[:, 0:1], ph[:, 0:1], ACT.Sigmoid)
        nc.scalar.activation(s_[:, 1:2], pg[:, 0:1], ACT.Sigmoid)
        nc.scalar.activation(s_[:, 2:3], pp[:, 0:1], ACT.Sigmoid)

        def tail_dve():
            pass

        def tail_fin():
            if li == 0:
                nc.scalar.activation(dst8[:, f, 0:1], s_[:, 0:1], ACT.Copy,
                                     scale=SH)
                nc.vector.tensor_copy(dst16[:, f, 0:1], s_[:, 0:1])
            else:
                nc.sync.dma_start(
                    y16[f * 128:(f + 1) * 128, i * TCE:(i + 1) * TCE], s_[:])
        return tail_dve, tail_fin
    nc.scalar.activation(s_[:], ph[:], ACT.Sigmoid)
    r_ = ew.tile([128, TCE], F16, tag="r")
    nc.scalar.activation(r_[:], ph[:], ACT.Relu)
    a_ = ew.tile([128, TCE], F16, tag="a")
    nc.scalar.activation(a_[:], pg[:], ACT.Sigmoid, scale=-INV8)
    ap_ = ew.tile([128, TCE], F16, tag="ap")
    nc.scalar.activation(ap_[:], pg[:], ACT.Sigmoid, scale=INV8)
    g_ = ew.tile([128, TCE], F16, tag="g")
    nc.scalar.activation(g_[:], pp[:], ACT.Sigmoid, scale=INV8)

    col = li * NFB + f
    if os.environ.get("ABLATE") == "1":
        # timing ablation: no DVE/Pool elementwise; cast/dst fed from s_
        def tail_dve():
            pass

        def tail_fin():
            if li == 0:
                nc.scalar.activation(dst8[:, f, :], s_[:], ACT.Copy, scale=SH)
                nc.vector.tensor_copy(dst16[:, f, :], s_[:])
            else:
                nc.sync.dma_start(
                    y16[f * 128:(f + 1) * 128, i * TCE:(i + 1) * TCE], s_[:])
        return tail_dve, tail_fin
    gh = ew.tile([128, TCE], F16, tag="gh")
    nc.vector.scalar_tensor_tensor(gh[:], s_[:], 0.5, r_[:],
                                   op0=ALU.min, op1=ALU.add)
    b_ = ew.tile([128, TCE], F16, tag="nb")
    nc.vector.tensor_tensor(b_[:], ap_[:], gh[:], op=ALU.mult)
    sc = ew.tile([128, TCE], F16, tag="sc")
    init = 0.0 if i == 0 else carries[:, col:col + 1]
    nc.vector.tensor_tensor_scan(sc[:], a_[:], b_[:], init,
                                 op0=ALU.mult, op1=ALU.add)
    if i < NSPAN - 1:
        nc.vector.tensor_copy(carries[:, col:col + 1], sc[:, TCE - 1:TCE])
    # Pool runs the whole highway chain in-order on one engine: no
    # cross-engine re-entry into the in-order ACT/DVE streams.
    hs = rhs16[:, f, :]
    d_ = ew.tile([128, TCE], F16, tag="d")
    if os.environ.get("POOL_DM", "1") == "1":
        nc.gpsimd.tensor_tensor(d_[:], sc[:], hs, op=ALU.subtract)
    else:
        nc.vector.tensor_tensor(d_[:], sc[:], hs, op=ALU.subtract)
    m_ = ew.tile([128, TCE], F16, tag="m", bufs=3)
    if os.environ.get("POOL_DM", "1") == "1":
        nc.gpsimd.tensor_tensor(m_[:], g_[:], d_[:], op=ALU.mult)
    else:
        nc.vector.tensor_tensor(m_[:], g_[:], d_[:], op=ALU.mult)

    def tail_dve():
        nc.vector.tensor_tensor(dst16[:, :] if li else dst16[:, f, :],
                                m_[:], hs, op=ALU.add)

    def tail_fin():
        if li == 0:
            nc.scalar.activation(dst8[:, f, :], dst16[:, f, :], ACT.Copy,
                                 scale=SH)
        else:
            nc.sync.dma_start(
                y16[f * 128:(f + 1) * 128, i * TCE:(i + 1) * TCE],
                dst16[:, :])

    return tail_dve, tail_fin


def _emit_body(tc_, y16, h16t, h8t, w16_sb, w8_sb, pools):
    nc = tc_.nc
    rhs_pool, ypool, psums, ew, carry_pool = pools
    carries = carry_pool.tile([128, 2 * NFB], F32)

    # Software-pipelined tails: the ACT fp8-cast / y-DMA runs 3 units
    # behind its producer so the in-order ACT/SP streams never block on the
    # scan->Pool highway chain.
    pend_dve = []
    pend_fin = []

    def emit(unit_args):
        # front first: the ACT sigmoids (which feed DVE) must precede the
        # delayed cast in the in-order ACT stream, else the loop-carried
        # cycle out->cast->s/r->gh->scan paces the whole kernel (~10us/unit
        # measured). Tag rotation stays safe: one allocation per unit per
        # tag, m has bufs=3, reads pop at most 2 units behind.
        td, tf = _emit_unit(*unit_args)
        if len(pend_dve) >= 2:
            pend_dve.pop(0)()
        if len(pend_fin) >= 2:
            pend_fin.pop(0)()
        pend_dve.append(td)
        pend_fin.append(tf)

    prev = None
    for i in range(NSPAN):
        rhs16 = rhs_pool.tile([128, NK, TCE], F16, tag="rhs16_l1")
        for k in range(NK):
            nc.gpsimd.dma_start(rhs16[:, k, :],
                                h16t[k * 128:(k + 1) * 128, i * TCE:(i + 1) * TCE])
        rhs8 = rhs_pool.tile([128, NK, TCE], F8, tag="rhs8_l1", bufs=1)
        for k in range(NK):
            nc.gpsimd.dma_start(rhs8[:, k, :],
                                h8t[k * 128:(k + 1) * 128, i * TCE:(i + 1) * TCE])
        out16 = rhs_pool.tile([128, NK, TCE], F16, tag="rhs16_l2")
        out8 = rhs_pool.tile([128, NK, TCE], F8, tag="rhs8_l2")
        if prev is None:
            for f in range(NFB):
                emit((nc, i, 0, f, w16_sb, w8_sb, rhs16, rhs8,
                      out16, out8, psums, ew, carries, None))
            # span 0 has no interleaved L2 units; flush so span 1's L2
            # matmuls see every span-0 cast already emitted
            for t in pend_dve:
                t()
            for t in pend_fin:
                t()
            pend_dve.clear()
            pend_fin.clear()
        else:
            (p16, p8) = prev
            for f in range(NFB):
                emit((nc, i, 0, f, w16_sb, w8_sb, rhs16, rhs8,
                      out16, out8, psums, ew, carries, None))
                ytile = ypool.tile([128, TCE], F16, tag="y", name="ytile")
                emit((nc, i - 1, 1, f, w16_sb, w8_sb, p16, p8,
                      ytile, None, psums, ew, carries, y16))
        prev = (out16, out8)
    (p16, p8) = prev
    # the final L2 block has no slack emit before its first unit: flush so
    # every span-3 cast/highway-out is emitted before L2 reads them
    for t in pend_dve:
        t()
    for t in pend_fin:
        t()
    pend_dve.clear()
    pend_fin.clear()
    for f in range(NFB):
        ytile = ypool.tile([128, TCE], F16, tag="y", name="ytile")
        emit((nc, NSPAN - 1, 1, f, w16_sb, w8_sb, p16, p8,
              ytile, None, psums, ew, carries, y16))
    for t in pend_dve:
        t()
    for t in pend_fin:
        t()


def build_nc(loop_iters: int = 1):
    """Build + compile the per-core Bass program (SPMD across 8 cores)."""
    nc = bacc.Bacc("TRN2", target_bir_lowering=False, debug=False,
                   enable_asserts=False, num_devices=8)
    h16t = nc.dram_tensor("h16t", [H, T], F16, kind="ExternalInput").ap()
    h8t = nc.dram_tensor("h8t", [H, T], F8, kind="ExternalInput").ap()
    w16 = nc.dram_tensor("w16", [2, NK, 128, C16], F16,
                         kind="ExternalInput").ap()
    w8 = nc.dram_tensor("w8", [2, NK, 128, C8], F8,
                        kind="ExternalInput").ap()
    y16 = nc.dram_tensor("y16", [H, T], F16, kind="ExternalOutput").ap()

    with tile.TileContext(nc) as tc_:
        with ExitStack() as ctx:
            wpool = ctx.enter_context(tc_.tile_pool(name="w", bufs=1))
            rhs_pool = ctx.enter_context(tc_.tile_pool(name="rhs", bufs=2))
            ypool = ctx.enter_context(tc_.tile_pool(name="y", bufs=2))
            psum_h = ctx.enter_context(
                tc_.tile_pool(name="psh", bufs=2, space="PSUM"))
            psum_g = ctx.enter_context(
                tc_.tile_pool(name="psg", bufs=1, space="PSUM"))
            psum_p = ctx.enter_context(
                tc_.tile_pool(name="psp", bufs=1, space="PSUM"))
            ew = ctx.enter_context(tc_.tile_pool(name="ew", bufs=2))
            carry_pool = ctx.enter_context(tc_.tile_pool(name="carry", bufs=1))

            w16_sb = []
            w8_sb = []
            for li in range(2):
                wt = wpool.tile([128, NK, C16], F16, tag=f"w16_{li}",
                                name=f"w16_{li}")
                for k in range(NK):
                    nc.gpsimd.dma_start(wt[:, k, :], w16[li, k])
                w16_sb.append(wt)
                wt8 = wpool.tile([128, NK, C8], F8, tag=f"w8_{li}",
                                 name=f"w8_{li}")
                for k in range(NK):
                    nc.gpsimd.dma_start(wt8[:, k, :], w8[li, k])
                w8_sb.append(wt8)

            # PE p-state warmup + ACT sigmoid table preload while the weight
            # stream is in flight. The warm matmuls write the proj psum tile
            # (reused by the first real unit afterwards).
            warm_in = ew.tile([128, TCE], F16, tag="gh", name="warm_in")
            nc.vector.memset(warm_in[:], 0.0)
            wp = psum_p.tile([128, TCE], F32, tag="pp", name="wp")
            for _ in range(24):
                nc.tensor.matmul(wp[:, 0:512], warm_in[:, 0:128],
                                 warm_in[:, 0:512], start=True, stop=True)
            warm_s = ew.tile([128, TCE], F16, tag="s", name="warm_s")
            nc.scalar.activation(warm_s[:, 0:1], wp[:, 0:1], ACT.Sigmoid)

            pools = (rhs_pool, ypool, (psum_h, psum_g, psum_p), ew, carry_pool)
            if loop_iters == 1:
                _emit_body(tc_, y16, h16t, h8t, w16_sb, w8_sb, pools)
            else:
                with tc_.For_i(0, loop_iters, 1):
                    _emit_body(tc_, y16, h16t, h8t, w16_sb, w8_sb, pools)
    nc.compile()
    return nc


_CACHED_NC = None


def _prep_inputs(h, W0, W1):
    e4 = ml_dtypes.float8_e4m3
    W = np.stack([np.asarray(W0, np.float32), np.asarray(W1, np.float32)])
    w16 = W[:, :, 0:H].reshape(2, NK, 128, C16)
    w8 = (W[:, :, H:] * SW).reshape(2, NK, 128, C8)
    base = {"w16": w16.astype(np.float16), "w8": w8.astype(e4)}
    maps = []
    for c in range(8):
        ht = np.ascontiguousarray(np.asarray(h[c]).T)
        m = dict(base)
        m["h16t"] = ht.astype(np.float16)
        m["h8t"] = (ht * SH).astype(e4)
        maps.append(m)
    return maps


def kernel(h, W0, W1):
    global _CACHED_NC
    if _CACHED_NC is None:
        _CACHED_NC = build_nc()
    res = bass_utils.run_bass_kernel_spmd(
        _CACHED_NC, _prep_inputs(h, W0, W1), core_ids=list(range(8)))
    return np.stack(
        [res.results[c]["y16"].T.astype(np.float32) for c in range(8)], axis=0)


# revision 42
# speedup vs baseline: 1.1063x; 1.0045x over previous
"""MinGRU (2-layer) Trainium2 Bass kernel.

Problem: h[8,4096,1024] f32, W0/W1 [1024,3072] f32.
Per layer: z = h @ W; hidden,gate,proj = split(z);
  a = sigmoid(-gate); g_hidden = relu(hidden) + min(sigmoid(hidden), 0.5)
  scan: out_t = a_t*out_{t-1} + (1-a_t)*g_hidden_t   (fp32 scan state)
  h' = sigmoid(proj)*out + (1-sigmoid(proj))*h

Sharding: one batch row per core (B=8 over 8 cores), weights replicated.

Design (engine-balanced against measured per-op HW costs):
  - hidden matmul fp16 (accuracy-critical path); gate/proj matmuls fp8 e4m3
    with DoubleRow perf mode (2 k-tiles per instruction, 2x PE throughput).
    Measured rel err 1.28e-2 vs the 2e-2 gate on the fixed harness inputs.
  - host pre-transposes h to [H,T] fp16+fp8(x8); y is written [H,T] fp16 and
    the host re-transposes + upcasts. No PE or DMA transposes on device.
  - PSUM tiles span 2 banks [128,1024]; ACT reads a full span in one op
    (1.18us vs 2x0.91us measured) and writes fp16 SBUF tiles.
  - elementwise split across engines per 1024-token span:
      ACT:  s=sig(hidden), r=relu(hidden), a=sig(-gate), ap=sig(gate),
            g=sig(proj), fp8 cast
      DVE:  gh=min(s,.5)+r, b=ap*gh (tt 2x), scan(a,b,op1=add), h'=m+h, carry
      Pool: d=sc-h, m=g*d
  - layer-2 interleaves with layer-1 one span behind; the DVE highway-out
    runs 2 units behind its producer and the ACT cast / y-DMA 2 units behind
    (popped after the DVE tail) so the in-order engine streams never block
    on the cross-engine scan->Pool chain.
  - input DMAs issue from the Pool queue (25ns/issue vs 565ns on SP).

Measured: ~630-650 us HW exec (baseline 914 us; shared-device load
adds up to +15% run-to-run), rel err 1.275e-2.
"""

import os
import sys

if "/opt/trn_rl_repo" not in sys.path:
    sys.path.insert(0, "/opt/trn_rl_repo")

from contextlib import ExitStack

import numpy as np
import ml_dtypes

import concourse.bass as bass
import concourse.tile as tile
from concourse import bacc, mybir
from concourse import bass_utils


T, H, H3 = 4096, 1024, 3072
TCE = 1024               # elementwise span (= 2 PSUM banks of fp32)
TSUB = 512               # DVE/Pool subtile
NSPAN = T // TCE
NFB = H // 128           # output feature blocks
NK = H // 128            # contraction k-tiles
F32 = mybir.dt.float32
F16 = mybir.dt.float16
F8 = mybir.dt.float8e4
ACT = mybir.ActivationFunctionType
ALU = mybir.AluOpType
DR = mybir.MatmulPerfMode.DoubleRow

SH = 8.0                 # fp8 activation scale
SW = 32.0                # fp8 weight scale
INV8 = 1.0 / (SH * SW)
C16 = H                  # fp16 weight cols per layer (hidden)
C8 = 2 * H               # fp8 weight cols per layer (gate, proj)


def _emit_unit(nc, i, li, f, w16_sb, w8_sb, rhs16, rhs8, dst16, dst8,
               psums, ew, carries, y16):
    """Emit matmuls + front elementwise for one (span, layer, f-block).

    Returns a closure emitting the tail (DVE highway-out, then ACT fp8 cast
    or the y DMA) which the caller schedules 1-2 units later so the in-order
    ACT/DVE streams never block on the cross-engine scan->Pool chain.
    """
    psum_h, psum_g, psum_p = psums
    ph = psum_h.tile([128, TCE], F32, tag="ph")
    pg = psum_g.tile([128, TCE], F32, tag="pg")
    pp = psum_p.tile([128, TCE], F32, tag="pp")
    w16 = w16_sb[li]
    w8 = w8_sb[li]
    for half in (0, 1):
        sl = slice(half * 512, (half + 1) * 512)
        for k in range(NK):
            nc.tensor.matmul(ph[:, sl], w16[:, k, f * 128:(f + 1) * 128],
                             rhs16[:, k, sl],
                             start=(k == 0), stop=(k == NK - 1))
        for k in range(0, NK, 2):
            nc.tensor.matmul(pg[:, sl], w8[:, k:k + 2, f * 128:(f + 1) * 128],
                             rhs8[:, k:k + 2, sl], perf_mode=DR,
                             start=(k == 0), stop=(k == NK - 2))
        for k in range(0, NK, 2):
            nc.tensor.matmul(pp[:, sl],
                             w8[:, k:k + 2, H + f * 128:H + (f + 1) * 128],
                             rhs8[:, k:k + 2, sl], perf_mode=DR,
                             start=(k == 0), stop=(k == NK - 2))
    # ACT: full-span transcendentals out of PSUM (s/r first: they gate the
    # next unit's hidden matmul group via psum reuse)
    s_ = ew.tile([128, TCE], F16, tag="s")
    if os.environ.get("ABLATE") == "2":
        nc.scalar.activation(s_[:, 0:1], ph[:, 0:1], ACT.Sigmoid)
        nc.scalar.activation(s_[:, 1:2], pg[:, 0:1], ACT.Sigmoid)
        nc.scalar.activation(s_[:, 2:3], pp[:, 0:1], ACT.Sigmoid)

        def tail_dve():
            pass

        def tail_fin():
            if li == 0:
                nc.scalar.activation(dst8[:, f, 0:1], s_[:, 0:1], ACT.Copy,
                                     scale=SH)
                nc.vector.tensor_copy(dst16[:, f, 0:1], s_[:, 0:1])
            else:
                nc.sync.dma_start(
                    y16[f * 128:(f + 1) * 128, i * TCE:(i + 1) * TCE], s_[:])
        return tail_dve, tail_fin
    nc.scalar.activation(s_[:], ph[:], ACT.Sigmoid)
    r_ = ew.tile([128, TCE], F16, tag="r")
    nc.scalar.activation(r_[:], ph[:], ACT.Relu)
    a_ = ew.tile([128, TCE], F16, tag="a")
    nc.scalar.activation(a_[:], pg[:], ACT.Sigmoid, scale=-INV8)
    ap_ = ew.tile([128, TCE], F16, tag="ap")
    nc.scalar.activation(ap_[:], pg[:], ACT.Sigmoid, scale=INV8)
    g_ = ew.tile([128, TCE], F16, tag="g")
    nc.scalar.activation(g_[:], pp[:], ACT.Sigmoid, scale=INV8)

    col = li * NFB + f
    if os.environ.get("ABLATE") == "1":
        # timing ablation: no DVE/Pool elementwise; cast/dst fed from s_
        def tail_dve():
            pass

        def tail_fin():
            if li == 0:
                nc.scalar.activation(dst8[:, f, :], s_[:], ACT.Copy, scale=SH)
                nc.vector.tensor_copy(dst16[:, f, :], s_[:])
            else:
                nc.sync.dma_start(
                    y16[f * 128:(f + 1) * 128, i * TCE:(i + 1) * TCE], s_[:])
        return tail_dve, tail_fin
    gh = ew.tile([128, TCE], F16, tag="gh")
    nc.vector.scalar_tensor_tensor(gh[:], s_[:], 0.5, r_[:],
                                   op0=ALU.min, op1=ALU.add)
    b_ = ew.tile([128, TCE], F16, tag="nb")
    nc.vector.tensor_tensor(b_[:], ap_[:], gh[:], op=ALU.mult)
    sc = ew.tile([128, TCE], F16, tag="sc")
    init = 0.0 if i == 0 else carries[:, col:col + 1]
    nc.vector.tensor_tensor_scan(sc[:], a_[:], b_[:], init,
                                 op0=ALU.mult, op1=ALU.add)
    if i < NSPAN - 1:
        nc.vector.tensor_copy(carries[:, col:col + 1], sc[:, TCE - 1:TCE])
    # Pool runs the whole highway chain in-order on one engine: no
    # cross-engine re-entry into the in-order ACT/DVE streams.
    hs = rhs16[:, f, :]
    d_ = ew.tile([128, TCE], F16, tag="d")
    if os.environ.get("POOL_DM", "1") == "1":
        nc.gpsimd.tensor_tensor(d_[:], sc[:], hs, op=ALU.subtract)
    else:
        nc.vector.tensor_tensor(d_[:], sc[:], hs, op=ALU.subtract)
    m_ = ew.tile([128, TCE], F16, tag="m", bufs=3)
    if os.environ.get("POOL_DM", "1") == "1":
        nc.gpsimd.tensor_tensor(m_[:], g_[:], d_[:], op=ALU.mult)
    else:
        nc.vector.tensor_tensor(m_[:], g_[:], d_[:], op=ALU.mult)

    def tail_dve():
        nc.vector.tensor_tensor(dst16[:, :] if li else dst16[:, f, :],
                                m_[:], hs, op=ALU.add)

    def tail_fin():
        if li == 0:
            nc.scalar.activation(dst8[:, f, :], dst16[:, f, :], ACT.Copy,
                                 scale=SH)
        else:
            nc.sync.dma_start(
                y16[f * 128:(f + 1) * 128, i * TCE:(i + 1) * TCE],
                dst16[:, :])

    return tail_dve, tail_fin


def _emit_body(tc_, y16, h16t, h8t, w16_sb, w8_sb, pools):
    nc = tc_.nc
    rhs_pool, ypool, psums, ew, carry_pool = pools
    carries = carry_pool.tile([128, 2 * NFB], F32)

    # Software-pipelined tails: the ACT fp8-cast / y-DMA runs 3 units
    # behind its producer so the in-order ACT/SP streams never block on the
    # scan->Pool highway chain.
    pend_dve = []
    pend_fin = []

    def emit(unit_args):
        # front first: the ACT sigmoids (which feed DVE) must precede the
        # delayed cast in the in-order ACT stream, else the loop-carried
        # cycle out->cast->s/r->gh->scan paces the whole kernel (~10us/unit
        # measured). Tag rotation stays safe: one allocation per unit per
        # tag, m has bufs=3, reads pop at most 2 units behind.
        td, tf = _emit_unit(*unit_args)
        if len(pend_dve) >= 2:
            pend_dve.pop(0)()
        if len(pend_fin) >= 2:
            pend_fin.pop(0)()
        pend_dve.append(td)
        pend_fin.append(tf)

    prev = None
    for i in range(NSPAN):
        rhs16 = rhs_pool.tile([128, NK, TCE], F16, tag="rhs16_l1")
        for k in range(NK):
            nc.gpsimd.dma_start(rhs16[:, k, :],
                                h16t[k * 128:(k + 1) * 128, i * TCE:(i + 1) * TCE])
        rhs8 = rhs_pool.tile([128, NK, TCE], F8, tag="rhs8_l1", bufs=1)
        for k in range(NK):
            nc.gpsimd.dma_start(rhs8[:, k, :],
                                h8t[k * 128:(k + 1) * 128, i * TCE:(i + 1) * TCE])
        out16 = rhs_pool.tile([128, NK, TCE], F16, tag="rhs16_l2")
        out8 = rhs_pool.tile([128, NK, TCE], F8, tag="rhs8_l2")
        if prev is None:
            for f in range(NFB):
                emit((nc, i, 0, f, w16_sb, w8_sb, rhs16, rhs8,
                      out16, out8, psums, ew, carries, None))
            # span 0 has no interleaved L2 units; flush so span 1's L2
            # matmuls see every span-0 cast already emitted
            for t in pend_dve:
                t()
            for t in pend_fin:
                t()
            pend_dve.clear()
            pend_fin.clear()
        else:
            (p16, p8) = prev
            for f in range(NFB):
                emit((nc, i, 0, f, w16_sb, w8_sb, rhs16, rhs8,
                      out16, out8, psums, ew, carries, None))
                ytile = ypool.tile([128, TCE], F16, tag="y", name="ytile")
                emit((nc, i - 1, 1, f, w16_sb, w8_sb, p16, p8,
                      ytile, None, psums, ew, carries, y16))
        prev = (out16, out8)
    (p16, p8) = prev
    # the final L2 block has no slack emit before its first unit: flush so
    # every span-3 cast/highway-out is emitted before L2 reads them
    for t in pend_dve:
        t()
    for t in pend_fin:
        t()
    pend_dve.clear()
    pend_fin.clear()
    for f in range(NFB):
        ytile = ypool.tile([128, TCE], F16, tag="y", name="ytile")
        emit((nc, NSPAN - 1, 1, f, w16_sb, w8_sb, p16, p8,
              ytile, None, psums, ew, carries, y16))
    for t in pend_dve:
        t()
    for t in pend_fin:
        t()


def build_nc(loop_iters: int = 1):
    """Build + compile the per-core Bass program (SPMD across 8 cores)."""
    nc = bacc.Bacc("TRN2", target_bir_lowering=False, debug=False,
                   enable_asserts=False, num_devices=8)
    h16t = nc.dram_tensor("h16t", [H, T], F16, kind="ExternalInput").ap()
    h8t = nc.dram_tensor("h8t", [H, T], F8, kind="ExternalInput").ap()
    w16 = nc.dram_tensor("w16", [2, NK, 128, C16], F16,
                         kind="ExternalInput").ap()
    w8 = nc.dram_tensor("w8", [2, NK, 128, C8], F8,
                        kind="ExternalInput").ap()
    y16 = nc.dram_tensor("y16", [H, T], F16, kind="ExternalOutput").ap()

    with tile.TileContext(nc) as tc_:
        with ExitStack() as ctx:
            wpool = ctx.enter_context(tc_.tile_pool(name="w", bufs=1))
            rhs_pool = ctx.enter_context(tc_.tile_pool(name="rhs", bufs=2))
            ypool = ctx.enter_context(tc_.tile_pool(name="y", bufs=2))
            psum_h = ctx.enter_context(
                tc_.tile_pool(name="psh", bufs=2, space="PSUM"))
            psum_g = ctx.enter_context(
                tc_.tile_pool(name="psg", bufs=1, space="PSUM"))
            psum_p = ctx.enter_context(
                tc_.tile_pool(name="psp", bufs=1, space="PSUM"))
            ew = ctx.enter_context(tc_.tile_pool(name="ew", bufs=2))
            carry_pool = ctx.enter_context(tc_.tile_pool(name="carry", bufs=1))

            w16_sb = []
            w8_sb = []
            for li in range(2):
                wt = wpool.tile([128, NK, C16], F16, tag=f"w16_{li}",
                                name=f"w16_{li}")
                for k in range(NK):
                    nc.gpsimd.dma_start(wt[:, k, :], w16[li, k])
                w16_sb.append(wt)
                wt8 = wpool.tile([128, NK, C8], F8, tag=f"w8_{li}",
                                 name=f"w8_{li}")
                for k in range(NK):
                    nc.gpsimd.dma_start(wt8[:, k, :], w8[li, k])
                w8_sb.append(wt8)

            # PE p-state warmup + ACT sigmoid table preload while the weight
            # stream is in flight. The warm matmuls write the proj psum tile
            # (reused by the first real unit afterwards).
            warm_in = ew.tile([128, TCE], F16, tag="gh", name="warm_in")
            nc.vector.memset(warm_in[:], 0.0)
            wp = psum_p.tile([128, TCE], F32, tag="pp", name="wp")
            for _ in range(24):
                nc.tensor.matmul(wp[:, 0:512], warm_in[:, 0:128],
                                 warm_in[:, 0:512], start=True, stop=True)
            warm_s = ew.tile([128, TCE], F16, tag="s", name="warm_s")
            nc.scalar.activation(warm_s[:, 0:1], wp[:, 0:1], ACT.Sigmoid)

            pools = (rhs_pool, ypool, (psum_h, psum_g, psum_p), ew, carry_pool)
            if loop_iters == 1:
                _emit_body(tc_, y16, h16t, h8t, w16_sb, w8_sb, pools)
            else:
                with tc_.For_i(0, loop_iters, 1):
                    _emit_body(tc_, y16, h16t, h8t, w16_sb, w8_sb, pools)
    nc.compile()
    return nc


_CACHED_NC = None


def _prep_inputs(h, W0, W1):
    e4 = ml_dtypes.float8_e4m3
    W = np.stack([np.asarray(W0, np.float32), np.asarray(W1, np.float32)])
    w16 = W[:, :, 0:H].reshape(2, NK, 128, C16)
    w8 = (W[:, :, H:] * SW).reshape(2, NK, 128, C8)
    base = {"w16": w16.astype(np.float16), "w8": w8.astype(e4)}
    maps = []
    for c in range(8):
        ht = np.ascontiguousarray(np.asarray(h[c]).T)
        m = dict(base)
        m["h16t"] = ht.astype(np.float16)
        m["h8t"] = (ht * SH).astype(e4)
        maps.append(m)
    return maps


def kernel(h, W0, W1):
    global _CACHED_NC
    if _CACHED_NC is None:
        _CACHED_NC = build_nc()
    res = bass_utils.run_bass_kernel_spmd(
        _CACHED_NC, _prep_inputs(h, W0, W1), core_ids=list(range(8)))
    return np.stack(
        [res.results[c]["y16"].T.astype(np.float32) for c in range(8)], axis=0)
